# revision 48
# baseline (speedup 1.0000x reference)
"""AttentionBlock Trainium2 kernel (v2: fp8 DoubleRow GEMMs).

Sharding: data-parallel over batch (B=8 -> one batch element per NeuronCore).

Per-core pipeline (C=512, HW=1024, 8 heads x 64):
  groupnorm (stats via indicator matmuls, f32)
  -> h cast to fp8 pair-tile layout [128,2,1024]
  -> QK GEMM in fp8 DoubleRow (2x MACs/cycle); biases algebraically removed:
       score bias terms ride as (a) one augmented contraction channel
       (tq = bk_h^T W_q,h h, paired with a constant-1 channel on the K side)
       and (b) a per-partition exp bias (tk = bq_h^T W_k,h h + bq.bk).
  -> Q,K shuffled to [33,2,1024] fp8 DR layout by on-chip DMA (c = 2p+j)
  -> scores GEMM fp8 DR, transposed [k,q]; exp with offset OFF so probs
     fit fp8e4 (max 240); denominators ride as a ones-column in the
     attn@v DR matmul; exp split: ScalarE native exp -> fp8, VectorE
     Schraudolph bit-trick (magic-add) -> bf16 bits -> GpSimd cast fp8.
  -> AV GEMM fp8 DR over kk-pairs -> normalize via reciprocal_approx_fast
     + DMA broadcast -> attO fp8 [64,2,1024] head-pair tiles
  -> proj GEMM fp8 DR + residual (+ W_p b_v + b_proj folded per-channel).
"""
import sys

sys.path.insert(0, "/opt/trn_rl_repo")
import numpy as np
import ml_dtypes
import concourse.bass as bass
import concourse.bacc as bacc
import concourse.tile as tile
from concourse import mybir
from concourse.bass_utils import run_bass_kernel_spmd

f32 = mybir.dt.float32
f32r = mybir.dt.float32r
bf16 = mybir.dt.bfloat16
fp8 = mybir.dt.float8e4
ALU = mybir.AluOpType
ACT = mybir.ActivationFunctionType
DR = mybir.MatmulPerfMode.DoubleRow

C = 512
HW = 1024
NH = 8
HD = 64
EPS = 1e-5
SCALE = HD ** -0.5
NT = C // 128  # 4 channel tiles
NP = HW // 128  # 8 position tiles

OFF = 2.0          # exp offset: probs = exp(s*SCALE - OFF)
A16 = 128.0 / np.log(2.0)   # Schraudolph slope for bf16 bit pattern
B16 = 127 * 128             # bf16 exponent bias << 7
CCOR = 9.0                  # Schraudolph bias correction
MAGIC = float(2 ** 23)
# kk tiles whose exp runs on DVE (Schraudolph) instead of ScalarE
EXP_DVE_KK = ()


def _build():
    nc = bacc.Bacc("TRN2", target_bir_lowering=False, debug=False, num_devices=8)
    x_d = nc.dram_tensor("x", [C, HW], f32, kind="ExternalInput").ap()
    wqk_d = nc.dram_tensor("wqk", [2, 128, 2, 2112], fp8, kind="ExternalInput").ap()
    cnsb_d = nc.dram_tensor("cnsb", [2, 1024], bf16, kind="ExternalInput").ap()
    onesr_d = nc.dram_tensor("onesr", [1, 8192], bf16, kind="ExternalInput").ap()
    wv_d = nc.dram_tensor("wv8", [2, 128, 2, 512], fp8, kind="ExternalInput").ap()
    wp_d = nc.dram_tensor("wp8", [4, 128, 2, 512], fp8, kind="ExternalInput").ap()
    wtld_d = nc.dram_tensor("wtld", [C], f32, kind="ExternalInput").ap()
    gamma_d = nc.dram_tensor("gamma", [C], f32, kind="ExternalInput").ap()
    beta_d = nc.dram_tensor("beta", [C], f32, kind="ExternalInput").ap()
    gind_d = nc.dram_tensor("gind", [128, 8], f32, kind="ExternalInput").ap()
    gbc_d = nc.dram_tensor("gbc", [8, 128], f32, kind="ExternalInput").ap()
    cba_d = nc.dram_tensor("cba", [64], f32, kind="ExternalInput").ap()
    cbs_d = nc.dram_tensor("cbs", [64], f32, kind="ExternalInput").ap()
    cns8_d = nc.dram_tensor("cns8", [2, 1024], fp8, kind="ExternalInput").ap()
    out_d = nc.dram_tensor("out", [C, HW], f32, kind="ExternalOutput").ap()
    eye8_d = nc.dram_tensor("eye8", [8, 8], f32, kind="ExternalInput").ap()
    ones64_d = nc.dram_tensor("ones64", [1, 64], f32, kind="ExternalInput").ap()
    d_scr = nc.dram_tensor("d_scr", [NH, HW], f32)

    with tile.TileContext(nc) as tc:
        with (
            tc.tile_pool(name="const", bufs=1) as cp,
            tc.tile_pool(name="gnp", bufs=2) as gnp,
            tc.tile_pool(name="xp", bufs=1) as xp,
            tc.tile_pool(name="hp", bufs=1) as hp,
            tc.tile_pool(name="qk8", bufs=1) as qk8p,
            tc.tile_pool(name="qs", bufs=1) as qsp,
            tc.tile_pool(name="vt", bufs=1) as vtp,
            tc.tile_pool(name="pp", bufs=8) as ppp,
            tc.tile_pool(name="pbf", bufs=2) as pbfp,
            tc.tile_pool(name="dsb", bufs=3) as dsbp,
            tc.tile_pool(name="rb", bufs=2) as rbp,
            tc.tile_pool(name="ao", bufs=1) as aop,
            tc.tile_pool(name="psA", bufs=2, space="PSUM") as psA,
            tc.tile_pool(name="psB", bufs=2, space="PSUM") as psB,
        ):
            # ---------------- loads ----------------
            xt = []
            for t in range(NT):
                xi = xp.tile([128, HW], f32, tag=f"x{t}")
                nc.sync.dma_start(out=xi, in_=x_d[t * 128:(t + 1) * 128, :])
                xt.append(xi)

            wqk = []
            for t in range(2):
                w = cp.tile([128, 2, 2112], fp8, tag=f"wqk{t}")
                nc.sync.dma_start(out=w, in_=wqk_d[t])
                wqk.append(w)
            wv = []
            for t in range(2):
                w = cp.tile([128, 2, 512], fp8, tag=f"wv{t}")
                nc.sync.dma_start(out=w, in_=wv_d[t])
                wv.append(w)
            wp = []
            for t in range(4):
                w = cp.tile([128, 2, 512], fp8, tag=f"wp{t}")
                nc.sync.dma_start(out=w, in_=wp_d[t])
                wp.append(w)

            def col_load(src_ap, offset, name):
                t_ = cp.tile([128, NT], f32, tag=name)
                nc.sync.dma_start(
                    out=t_,
                    in_=bass.AP(tensor=src_ap.tensor, offset=offset,
                                ap=[[1, 128], [128, NT]]),
                )
                return t_

            gamma_sb = col_load(gamma_d, 0, "gamma")
            beta_sb = col_load(beta_d, 0, "beta")
            wtld_sb = col_load(wtld_d, 0, "wtld")

            cba_sb = cp.tile([128, 64], f32, tag="cba")
            nc.sync.dma_start(
                out=cba_sb,
                in_=bass.AP(tensor=cba_d.tensor, offset=0, ap=[[0, 128], [1, 64]]))
            cbs_sb = cp.tile([128, 64], f32, tag="cbs")
            nc.sync.dma_start(
                out=cbs_sb,
                in_=bass.AP(tensor=cbs_d.tensor, offset=0, ap=[[0, 128], [1, 64]]))

            o64b = cp.tile([65, 64], bf16, tag="o64b")
            nc.sync.dma_start(out=o64b[64:65, :], in_=cnsb_d[0:1, 0:64])
            i8f = cp.tile([40, 8], f32, tag="i8f")
            nc.sync.dma_start(out=i8f[32:40, :], in_=eye8_d)
            gind_f = cp.tile([128, 8], f32, tag="gindf")
            nc.sync.dma_start(out=gind_f, in_=gind_d)
            gbc_f = cp.tile([8, 128], f32, tag="gbcf")
            nc.sync.dma_start(out=gbc_f, in_=gbc_d)

            # Q/K bf16 scores operands [rows, head, pos], 128-row padded
            # so the scores contraction lights the full PE array (HAM warm).
            # QsB rows: q(0-63), zeros(64-126), tq(127);
            # KsB rows: k(0-63), zeros(64-126), ones(127).
            QsB = qsp.tile([128, NH, HW], bf16, tag="QsB")
            KsB = qsp.tile([128, NH, HW], bf16, tag="KsB")
            nc.sync.dma_start(out=KsB[127:128, :, :], in_=onesr_d)

            # vT tiles per kk-pair: [128, 2, NH, 96-padded]; col 64 = ones.
            ones16f = cp.tile([128, 16], f32, tag="ones16f")
            nc.gpsimd.memset(ones16f, 1.0)
            vt = []
            for kp in range(4):
                v = vtp.tile([128, 2, NH, 96], fp8, tag=f"vt{kp}")
                nc.gpsimd.tensor_copy(
                    out=v[:, :, :, HD:HD + 1].rearrange("p a b c -> p (a b c)"),
                    in_=ones16f)
                vt.append(v)

            # ---------------- groupnorm ----------------
            eps_t = cp.tile([128, 1], f32, tag="eps")
            nc.vector.memset(eps_t, EPS)
            pg = psB.tile([128, 1024], f32, tag="B", name="pg")
            mes = []
            for t in range(NT):
                st = gnp.tile([128, 2, 6], f32, tag="bnst")
                nc.vector.bn_stats(out=st[:, 0, :], in_=xt[t][:, 0:512])
                nc.vector.bn_stats(out=st[:, 1, :], in_=xt[t][:, 512:1024])
                mv = gnp.tile([128, 2], f32, tag="mv")
                nc.vector.bn_aggr(out=mv, in_=st)
                me = gnp.tile([128, 2], f32r, tag=f"me{t}", name=f"me{t}")
                nc.vector.tensor_copy(out=me[:, 0:1], in_=mv[:, 0:1])
                sq = gnp.tile([128, 1], f32, tag="sq")
                nc.vector.tensor_mul(out=sq, in0=mv[:, 0:1], in1=mv[:, 0:1])
                nc.vector.tensor_add(out=me[:, 1:2], in0=mv[:, 1:2], in1=sq)
                mes.append(me)
            gind_r = cp.tile([128, 8], f32r, tag="gindr")
            nc.vector.tensor_copy(out=gind_r, in_=gind_f)
            gbc_r = cp.tile([8, 128], f32r, tag="gbcr")
            nc.vector.tensor_copy(out=gbc_r, in_=gbc_f)
            i8r = cp.tile([40, 8], f32r, tag="i8r")
            nc.vector.tensor_copy(out=i8r[32:40, :], in_=i8f[32:40, :])
            for t in range(NT):
                nc.tensor.matmul(
                    pg[0:8, 0:8].rearrange("p (a b) -> p a b", b=2)[:, t, :],
                    lhsT=gind_r, rhs=mes[t], start=True, stop=True)
            pgv = pg[0:8, 0:8].rearrange("p (a b) -> p a b", b=2)
            mE = gnp.tile([8, 4, 2], f32, tag="mE")
            nc.vector.tensor_scalar_mul(out=mE, in0=pgv, scalar1=1.0 / 16.0)
            var_t = gnp.tile([8, 4], f32, tag="var")
            nc.vector.tensor_mul(out=var_t, in0=mE[:, :, 0], in1=mE[:, :, 0])
            nc.vector.tensor_sub(out=var_t, in0=mE[:, :, 1], in1=var_t)
            sd = gnp.tile([8, 4], f32, tag="sd")
            nc.scalar.activation(out=sd, in_=var_t, func=ACT.Sqrt,
                                 bias=eps_t[0:8, :], scale=1.0)
            m_rs = gnp.tile([8, 4, 2], f32r, tag="m_rs")
            nc.vector.tensor_copy(out=m_rs[:, :, 0], in_=mE[:, :, 0])
            with nc.allow_low_precision(reason="f32r rstd for matmul broadcast"):
                nc.vector.reciprocal(out=m_rs[:, :, 1], in_=sd)

            # h fp8 pair tiles [128, 2, 1024]; channel c = 256t + 128j + p
            ht = [hp.tile([128, 2, HW], fp8, tag=f"h{t}", name=f"h{t}")
                  for t in range(2)]
            for ct in range(NT):
                bc_ps = psB.tile([128, 1024], f32, tag="B", name=f"bc{ct}")
                nc.tensor.matmul(bc_ps[:, 0:2], lhsT=gbc_r, rhs=m_rs[:, ct, :],
                                 start=True, stop=True)
                mrt = gnp.tile([128, 2], f32, tag="mrt")
                nc.vector.tensor_copy(out=mrt, in_=bc_ps[:, 0:2])
                A_t = gnp.tile([128, 1], f32, tag=f"A{ct}", name=f"A{ct}")
                nc.vector.tensor_mul(out=A_t, in0=gamma_sb[:, ct:ct + 1],
                                     in1=mrt[:, 1:2])
                B_t = gnp.tile([128, 1], f32, tag=f"B{ct}", name=f"B{ct}")
                tmb = gnp.tile([128, 1], f32, tag="tmb")
                nc.vector.tensor_mul(out=tmb, in0=mrt[:, 0:1], in1=A_t)
                nc.vector.tensor_sub(out=B_t, in0=beta_sb[:, ct:ct + 1], in1=tmb)
                eng = nc.vector if ct % 2 == 0 else nc.gpsimd
                eng.tensor_scalar(
                    out=ht[ct // 2][:, ct % 2, :], in0=xt[ct],
                    scalar1=A_t, scalar2=B_t, op0=ALU.mult, op1=ALU.add)

            # ---------------- QK GEMM (fp8 DR) ----------------
            # W free layout: Q_h at 96*(2h) (65 rows: q+tq), K_h at
            # 96*(2h+1) (64 rows), aug tk at 1536 (40 rows, tk at 32-39).
            tkf = cp.tile([40, HW], f32, tag="tkf")
            blocks = []
            for h in range(NH):
                blocks.append((128 * (2 * h), 128, ("q", h)))
                blocks.append((128 * (2 * h + 1), 128, ("k", h)))
            blocks.append((2048, 40, ("aug", 0)))
            for off, rows, (kind, h) in blocks:
                ps = psA.tile([128, 1024], f32, tag="A", name=f"qk{kind}{h}")
                for t in range(2):
                    for n in range(2):
                        nc.tensor.matmul(
                            ps[0:rows, n * 512:(n + 1) * 512],
                            lhsT=wqk[t][:, :, off:off + rows],
                            rhs=ht[t][:, :, n * 512:(n + 1) * 512],
                            start=(t == 0), stop=(t == 1), perf_mode=DR)
                if kind == "q":
                    nc.vector.tensor_copy(out=QsB[:, h, :], in_=ps)
                elif kind == "k":
                    nc.scalar.copy(out=KsB[0:127, h, :], in_=ps[0:127, :])
                else:
                    nc.vector.tensor_copy(out=tkf[32:40, :], in_=ps[32:40, :])

            # tkT [128, 64]: tkT[p, kk*8+h] = tk_h[kk*128+p] via PE transpose
            tkr = cp.tile([40, HW], f32r, tag="tkr")
            nc.vector.tensor_copy(out=tkr[32:40, :], in_=tkf[32:40, :])
            tkps = psA.tile([128, 1024], f32, tag="A", name="tkps")
            for kk in range(NP):
                nc.tensor.matmul(
                    tkps[:, kk * 8:(kk + 1) * 8],
                    lhsT=tkr[32:40, kk * 128:(kk + 1) * 128],
                    rhs=i8r[32:40, :], start=True, stop=True)
            tkT = cp.tile([128, 64], f32, tag="tkT")
            nc.vector.tensor_copy(out=tkT, in_=tkps[:, 0:64])
            # exp biases: actB = SCALE*tk + (SCALE*bqbk - OFF)
            actB = cp.tile([128, 64], f32, tag="actB")
            nc.vector.scalar_tensor_tensor(out=actB, in0=tkT, scalar=SCALE,
                                           in1=cba_sb, op0=ALU.mult, op1=ALU.add)
            # Schraudolph bias: schB = A16*SCALE*tk + cbs
            schB = cp.tile([128, 64], f32, tag="schB")
            nc.vector.scalar_tensor_tensor(out=schB, in0=tkT, scalar=A16 * SCALE,
                                           in1=cbs_sb, op0=ALU.mult, op1=ALU.add)

            # ---------------- V GEMM (fp8 DR) ----------------
            for kk in range(NP):
                ps = psB.tile([128, 1024], f32, tag="B", name=f"v{kk}")
                for t in range(2):
                    nc.tensor.matmul(
                        ps[:, 0:512],
                        lhsT=ht[t][:, :, kk * 128:(kk + 1) * 128],
                        rhs=wv[t], start=(t == 0), stop=(t == 1), perf_mode=DR)
                nc.vector.tensor_copy(
                    out=vt[kk // 2][:, kk % 2, :, 0:HD],
                    in_=ps[:, 0:512].rearrange("p (h d) -> p h d", h=NH))

            # ---------------- attention ----------------
            attO = [aop.tile([128, 2, HW], fp8, tag=f"ao{tp}", name=f"ao{tp}")
                    for tp in range(4)]
            for tp in range(4):
                nc.gpsimd.memset(attO[tp][64:128, :, :], 0.0)

            def emit_scores(h):
                pps = []
                for kp in range(4):
                    pps.append(ppp.tile([128, 2, HW], fp8, tag="pp",
                                        name=f"pp{h}_{kp}"))
                pb3 = pbfp.tile([128, HW], bf16, tag="pb3", name=f"pb3{h}")
                for kk in range(NP):
                    ps = psA.tile([128, 1024], f32, tag="A", name=f"sc{h}_{kk}")
                    for n in range(2):
                        nc.tensor.matmul(
                            ps[:, n * 512:(n + 1) * 512],
                            lhsT=KsB[:, h, kk * 128:(kk + 1) * 128],
                            rhs=QsB[:, h, n * 512:(n + 1) * 512],
                            start=True, stop=True)
                    col = kk * 8 + h
                    if kk in EXP_DVE_KK:
                        # Schraudolph: int16 bits of bf16 exp approximation,
                        # then contiguous bf16 -> fp8 cast on GpSimd
                        nc.vector.tensor_scalar(
                            out=pb3.bitcast(mybir.dt.int16),
                            in0=ps, scalar1=A16 * SCALE,
                            scalar2=schB[:, col:col + 1],
                            op0=ALU.mult, op1=ALU.add)
                        nc.gpsimd.tensor_copy(
                            out=pps[kk // 2][:, kk % 2, :], in_=pb3)
                    else:
                        nc.scalar.activation(
                            out=pps[kk // 2][:, kk % 2, :], in_=ps, func=ACT.Exp,
                            bias=actB[:, col:col + 1], scale=SCALE)
                return pps

            def emit_av(h, pps):
                pa = psB.tile([128, 1024], f32, tag="B", name=f"pa{h}")
                for kp in range(4):
                    for n in range(2):
                        nc.tensor.matmul(
                            pa[0:HD + 1, n * 512:(n + 1) * 512],
                            lhsT=vt[kp][:, :, h, 0:HD + 1],
                            rhs=pps[kp][:, :, n * 512:(n + 1) * 512],
                            start=(kp == 0), stop=(kp == 3), perf_mode=DR)
                # evacuate unnormalized AV + denominator row to SBUF bf16
                paS = dsbp.tile([65, HW], bf16, tag="paS", name=f"paS{h}")
                with nc.allow_low_precision(reason="bf16 raw attn out"):
                    nc.vector.tensor_copy(out=paS, in_=pa[0:65, :])
                return paS

            def emit_norm(h, paS):
                rbps = psB.tile([128, 1024], f32, tag="B", name=f"rbps{h}")
                for n in range(2):
                    nc.tensor.matmul(
                        rbps[0:64, n * 512:(n + 1) * 512],
                        lhsT=o64b[64:65, :],
                        rhs=paS[64:65, n * 512:(n + 1) * 512],
                        start=True, stop=True)
                rb = rbp.tile([64, HW], f32, tag="rb", name=f"rb{h}")
                with nc.allow_low_precision(reason="recip of softmax denom"):
                    nc.vector.reciprocal_approx_fast(
                        out=rb, in_=rbps[0:64, :])
                nc.vector.tensor_mul(
                    out=attO[h // 2][0:64, h % 2, :], in0=paS[0:64, :], in1=rb)

            prev = emit_scores(0)
            pend = None
            for h in range(NH):
                nxt = emit_scores(h + 1) if h + 1 < NH else None
                cur = emit_av(h, prev)
                if pend is not None:
                    emit_norm(h - 1, pend)
                pend = cur
                prev = nxt
            emit_norm(NH - 1, pend)

            # ---------------- proj + residual ----------------
            for m in range(NT):
                ps = psA.tile([128, 1024], f32, tag="A", name=f"pr{m}")
                for tp in range(4):
                    for n in range(2):
                        nc.tensor.matmul(
                            ps[:, n * 512:(n + 1) * 512],
                            lhsT=wp[tp][:, :, m * 128:(m + 1) * 128],
                            rhs=attO[tp][:, :, n * 512:(n + 1) * 512],
                            start=(tp == 0), stop=(tp == 3), perf_mode=DR)
                nc.vector.scalar_tensor_tensor(
                    out=xt[m], in0=ps, scalar=wtld_sb[:, m:m + 1],
                    in1=xt[m], op0=ALU.add, op1=ALU.add)
                nc.sync.dma_start(out=out_d[m * 128:(m + 1) * 128, :], in_=xt[m])
    nc.compile()
    return nc


def _prep_common(gamma, beta, w_qkv, b_qkv, w_proj, b_proj):
    fp8np = ml_dtypes.float8_e4m3fn
    w_qkv = np.asarray(w_qkv, np.float32)
    w_proj = np.asarray(w_proj, np.float32)
    b_qkv = np.asarray(b_qkv, np.float32)
    b_proj = np.asarray(b_proj, np.float32)
    wq, wk, wvm = w_qkv[0:C], w_qkv[C:2 * C], w_qkv[2 * C:3 * C]
    bq, bk, bv = b_qkv[0:C], b_qkv[C:2 * C], b_qkv[2 * C:3 * C]

    # W free layout per head: Q-block [q(64); 0(63); tq(1)] at 128*(2h),
    # K-block [k(64); 0(64)] at 128*(2h+1); aug tk rows 32-39 at 2048.
    Wqk = np.zeros((2112, C), np.float32)
    for h in range(NH):
        sl = slice(h * HD, (h + 1) * HD)
        qo = 128 * (2 * h)
        ko = 128 * (2 * h + 1)
        Wqk[qo:qo + 64] = wq[sl]
        Wqk[qo + 127] = bk[sl] @ wq[sl]
        Wqk[ko:ko + 64] = wk[sl]
        Wqk[2048 + 32 + h] = bq[sl] @ wk[sl]
    WqkT = Wqk.T  # [512 c, 2112]
    wqk_dr = np.stack([
        WqkT[256 * t:256 * (t + 1)].reshape(2, 128, 2112).transpose(1, 0, 2)
        for t in range(2)])  # [2, 128, 2, 2112]

    WvT = wvm.T  # [512 c, 512 oc]
    wv_dr = np.stack([
        WvT[256 * t:256 * (t + 1)].reshape(2, 128, 512).transpose(1, 0, 2)
        for t in range(2)])

    WpT = w_proj.T  # [512 c, 512 m]
    wp_dr = np.zeros((4, 128, 2, 512), np.float32)
    for tp in range(4):
        wp_dr[tp, 0:64] = (
            WpT[128 * tp:128 * (tp + 1)].reshape(2, 64, 512).transpose(1, 0, 2))

    wtld = w_proj @ bv + b_proj

    bqbk = np.array([bq[h * HD:(h + 1) * HD] @ bk[h * HD:(h + 1) * HD]
                     for h in range(NH)], np.float32)
    cba = np.tile(SCALE * bqbk - OFF, 8).astype(np.float32)
    cbs = np.tile(A16 * (SCALE * bqbk - OFF) + (B16 - CCOR),
                  8).astype(np.float32)

    cns8 = np.zeros((2, 1024), fp8np)
    cns8[0] = fp8np(1.0)

    return {
        "wqk": np.ascontiguousarray(wqk_dr.astype(fp8np)),
        "wv8": np.ascontiguousarray(wv_dr.astype(fp8np)),
        "wp8": np.ascontiguousarray(wp_dr.astype(fp8np)),
        "wtld": np.ascontiguousarray(wtld.astype(np.float32)),
        "gamma": np.ascontiguousarray(np.asarray(gamma, np.float32)),
        "beta": np.ascontiguousarray(np.asarray(beta, np.float32)),
        "gind": np.ascontiguousarray(
            np.repeat(np.eye(8, dtype=np.float32), 16, axis=0)),
        "gbc": np.ascontiguousarray(
            np.repeat(np.eye(8, dtype=np.float32), 16, axis=1)),
        "cba": cba,
        "cbs": cbs,
        "cns8": cns8,
        "cnsb": np.concatenate([np.ones((1, 1024), ml_dtypes.bfloat16),
                                np.zeros((1, 1024), ml_dtypes.bfloat16)]),
        "onesr": np.ones((1, 8192), ml_dtypes.bfloat16),
        "eye8": np.eye(8, dtype=np.float32),
        "ones64": np.ones((1, 64), np.float32),
    }


_NC = None


def kernel(x, gamma, beta, w_qkv, b_qkv, w_proj, b_proj):
    global _NC
    x = np.asarray(x, dtype=np.float32)
    B = x.shape[0]
    assert B == 8
    if _NC is None:
        _NC = _build()
    common = _prep_common(gamma, beta, w_qkv, b_qkv, w_proj, b_proj)
    in_maps = [
        {"x": np.ascontiguousarray(x[b].reshape(C, HW)), **common}
        for b in range(B)
    ]
    res = run_bass_kernel_spmd(_NC, in_maps, core_ids=list(range(8)))
    out = np.stack([res.results[b]["out"] for b in range(B)])
    return out.reshape(B, C, 32, 32).astype(np.float32)


# revision 49
# speedup vs baseline: 1.1734x; 1.1734x over previous
"""AttentionBlock Trainium2 kernel (v2: fp8 DoubleRow GEMMs).

Sharding: data-parallel over batch (B=8 -> one batch element per NeuronCore).

Per-core pipeline (C=512, HW=1024, 8 heads x 64):
  groupnorm (stats via indicator matmuls, f32)
  -> h cast to fp8 pair-tile layout [128,2,1024]
  -> QK GEMM in fp8 DoubleRow (2x MACs/cycle); biases algebraically removed:
       score bias terms ride as (a) one augmented contraction channel
       (tq = bk_h^T W_q,h h, paired with a constant-1 channel on the K side)
       and (b) a per-partition exp bias (tk = bq_h^T W_k,h h + bq.bk).
  -> Q,K shuffled to [33,2,1024] fp8 DR layout by on-chip DMA (c = 2p+j)
  -> scores GEMM fp8 DR, transposed [k,q]; exp with offset OFF so probs
     fit fp8e4 (max 240); denominators ride as a ones-column in the
     attn@v DR matmul; exp split: ScalarE native exp -> fp8, VectorE
     Schraudolph bit-trick (magic-add) -> bf16 bits -> GpSimd cast fp8.
  -> AV GEMM fp8 DR over kk-pairs -> normalize via reciprocal_approx_fast
     + DMA broadcast -> attO fp8 [64,2,1024] head-pair tiles
  -> proj GEMM fp8 DR + residual (+ W_p b_v + b_proj folded per-channel).
"""
import sys

sys.path.insert(0, "/opt/trn_rl_repo")
import numpy as np
import ml_dtypes
import concourse.bass as bass
import concourse.bacc as bacc
import concourse.tile as tile
from concourse import mybir
from concourse.bass_utils import run_bass_kernel_spmd

f32 = mybir.dt.float32
f32r = mybir.dt.float32r
bf16 = mybir.dt.bfloat16
fp8 = mybir.dt.float8e4
ALU = mybir.AluOpType
ACT = mybir.ActivationFunctionType
DR = mybir.MatmulPerfMode.DoubleRow

C = 512
HW = 1024
NH = 8
HD = 64
EPS = 1e-5
SCALE = HD ** -0.5
NT = C // 128  # 4 channel tiles
NP = HW // 128  # 8 position tiles

OFF = 2.0          # exp offset: probs = exp(s*SCALE - OFF)
A16 = 128.0 / np.log(2.0)   # Schraudolph slope for bf16 bit pattern
B16 = 127 * 128             # bf16 exponent bias << 7
CCOR = 9.0                  # Schraudolph bias correction
MAGIC = float(2 ** 23)
# kk tiles whose exp runs on DVE (Schraudolph) instead of ScalarE
EXP_DVE_KK = ()


def _build():
    nc = bacc.Bacc("TRN2", target_bir_lowering=False, debug=False, num_devices=8)
    x_d = nc.dram_tensor("x", [C, HW], f32, kind="ExternalInput").ap()
    wqk_d = nc.dram_tensor("wqk", [2, 128, 2, 2112], fp8, kind="ExternalInput").ap()
    cnsb_d = nc.dram_tensor("cnsb", [2, 1024], bf16, kind="ExternalInput").ap()
    onesr_d = nc.dram_tensor("onesr", [1, 8192], bf16, kind="ExternalInput").ap()
    wv_d = nc.dram_tensor("wv8", [2, 128, 2, 512], fp8, kind="ExternalInput").ap()
    wp_d = nc.dram_tensor("wp8", [4, 128, 2, 512], fp8, kind="ExternalInput").ap()
    wtld_d = nc.dram_tensor("wtld", [C], f32, kind="ExternalInput").ap()
    gamma_d = nc.dram_tensor("gamma", [C], f32, kind="ExternalInput").ap()
    beta_d = nc.dram_tensor("beta", [C], f32, kind="ExternalInput").ap()
    gind_d = nc.dram_tensor("gind", [128, 8], f32, kind="ExternalInput").ap()
    gbc_d = nc.dram_tensor("gbc", [8, 128], f32, kind="ExternalInput").ap()
    cba_d = nc.dram_tensor("cba", [64], f32, kind="ExternalInput").ap()
    cbs_d = nc.dram_tensor("cbs", [64], f32, kind="ExternalInput").ap()
    cns8_d = nc.dram_tensor("cns8", [2, 1024], fp8, kind="ExternalInput").ap()
    out_d = nc.dram_tensor("out", [C, HW], f32, kind="ExternalOutput").ap()
    eye8_d = nc.dram_tensor("eye8", [8, 8], f32, kind="ExternalInput").ap()
    ones64_d = nc.dram_tensor("ones64", [1, 64], f32, kind="ExternalInput").ap()
    d_scr = nc.dram_tensor("d_scr", [NH, HW], f32)

    with tile.TileContext(nc) as tc:
        with (
            tc.tile_pool(name="const", bufs=1) as cp,
            tc.tile_pool(name="gnp", bufs=2) as gnp,
            tc.tile_pool(name="xp", bufs=1) as xp,
            tc.tile_pool(name="hp", bufs=1) as hp,
            tc.tile_pool(name="qk8", bufs=1) as qk8p,
            tc.tile_pool(name="qs", bufs=1) as qsp,
            tc.tile_pool(name="vt", bufs=1) as vtp,
            tc.tile_pool(name="pp", bufs=8) as ppp,
            tc.tile_pool(name="pbf", bufs=2) as pbfp,
            tc.tile_pool(name="dsb", bufs=3) as dsbp,
            tc.tile_pool(name="rb", bufs=2) as rbp,
            tc.tile_pool(name="ao", bufs=1) as aop,
            tc.tile_pool(name="psA", bufs=2, space="PSUM") as psA,
            tc.tile_pool(name="psB", bufs=2, space="PSUM") as psB,
        ):
            # ---------------- loads ----------------
            xt = []
            for t in range(NT):
                xi = xp.tile([128, HW], f32, tag=f"x{t}")
                nc.sync.dma_start(out=xi, in_=x_d[t * 128:(t + 1) * 128, :])
                xt.append(xi)

            wqk = []
            for t in range(2):
                w = cp.tile([128, 2, 2112], fp8, tag=f"wqk{t}")
                nc.sync.dma_start(out=w, in_=wqk_d[t])
                wqk.append(w)
            wv = []
            for t in range(2):
                w = cp.tile([128, 2, 512], fp8, tag=f"wv{t}")
                nc.sync.dma_start(out=w, in_=wv_d[t])
                wv.append(w)
            wp = []
            for t in range(4):
                w = cp.tile([128, 2, 512], fp8, tag=f"wp{t}")
                nc.sync.dma_start(out=w, in_=wp_d[t])
                wp.append(w)

            def col_load(src_ap, offset, name):
                t_ = cp.tile([128, NT], f32, tag=name)
                nc.sync.dma_start(
                    out=t_,
                    in_=bass.AP(tensor=src_ap.tensor, offset=offset,
                                ap=[[1, 128], [128, NT]]),
                )
                return t_

            gamma_sb = col_load(gamma_d, 0, "gamma")
            beta_sb = col_load(beta_d, 0, "beta")
            wtld_sb = col_load(wtld_d, 0, "wtld")

            cba_sb = cp.tile([128, 64], f32, tag="cba")
            nc.sync.dma_start(
                out=cba_sb,
                in_=bass.AP(tensor=cba_d.tensor, offset=0, ap=[[0, 128], [1, 64]]))
            cbs_sb = cp.tile([128, 64], f32, tag="cbs")
            nc.sync.dma_start(
                out=cbs_sb,
                in_=bass.AP(tensor=cbs_d.tensor, offset=0, ap=[[0, 128], [1, 64]]))

            o64b = cp.tile([65, 64], bf16, tag="o64b")
            nc.sync.dma_start(out=o64b[64:65, :], in_=cnsb_d[0:1, 0:64])
            i8f = cp.tile([40, 8], f32, tag="i8f")
            nc.sync.dma_start(out=i8f[32:40, :], in_=eye8_d)
            gind_f = cp.tile([128, 8], f32, tag="gindf")
            nc.sync.dma_start(out=gind_f, in_=gind_d)
            gbc_f = cp.tile([8, 128], f32, tag="gbcf")
            nc.sync.dma_start(out=gbc_f, in_=gbc_d)

            # Q/K bf16 scores operands [rows, head, pos], 128-row padded
            # so the scores contraction lights the full PE array (HAM warm).
            # QsB rows: q(0-63), zeros(64-126), tq(127);
            # KsB rows: k(0-63), zeros(64-126), ones(127).
            QsB = qsp.tile([128, NH, HW], bf16, tag="QsB")
            KsB = qsp.tile([128, NH, HW], bf16, tag="KsB")
            nc.sync.dma_start(out=KsB[127:128, :, :], in_=onesr_d)

            # vT tiles per kk-pair: [128, 2, NH, 96-padded]; col 64 = ones.
            ones16f = cp.tile([128, 16], f32, tag="ones16f")
            nc.gpsimd.memset(ones16f, 1.0)
            vt = []
            for kp in range(4):
                v = vtp.tile([128, 2, NH, 96], fp8, tag=f"vt{kp}")
                nc.vector.tensor_copy(
                    out=v[:, :, :, HD:HD + 1].rearrange("p a b c -> p (a b c)"),
                    in_=ones16f)
                vt.append(v)

            # ---------------- groupnorm ----------------
            eps_t = cp.tile([128, 1], f32, tag="eps")
            nc.vector.memset(eps_t, EPS)
            pg = psB.tile([128, 1024], f32, tag="B", name="pg")
            mes = []
            for t in range(NT):
                st = gnp.tile([128, 2, 6], f32, tag="bnst")
                nc.vector.bn_stats(out=st[:, 0, :], in_=xt[t][:, 0:512])
                nc.vector.bn_stats(out=st[:, 1, :], in_=xt[t][:, 512:1024])
                mv = gnp.tile([128, 2], f32, tag="mv")
                nc.vector.bn_aggr(out=mv, in_=st)
                me = gnp.tile([128, 2], f32r, tag=f"me{t}", name=f"me{t}")
                nc.vector.tensor_copy(out=me[:, 0:1], in_=mv[:, 0:1])
                sq = gnp.tile([128, 1], f32, tag="sq")
                nc.vector.tensor_mul(out=sq, in0=mv[:, 0:1], in1=mv[:, 0:1])
                nc.vector.tensor_add(out=me[:, 1:2], in0=mv[:, 1:2], in1=sq)
                mes.append(me)
            gind_r = cp.tile([128, 8], f32r, tag="gindr")
            nc.vector.tensor_copy(out=gind_r, in_=gind_f)
            gbc_r = cp.tile([8, 128], f32r, tag="gbcr")
            nc.vector.tensor_copy(out=gbc_r, in_=gbc_f)
            i8r = cp.tile([40, 8], f32r, tag="i8r")
            nc.vector.tensor_copy(out=i8r[32:40, :], in_=i8f[32:40, :])
            for t in range(NT):
                nc.tensor.matmul(
                    pg[0:8, 0:8].rearrange("p (a b) -> p a b", b=2)[:, t, :],
                    lhsT=gind_r, rhs=mes[t], start=True, stop=True)
            pgv = pg[0:8, 0:8].rearrange("p (a b) -> p a b", b=2)
            mE = gnp.tile([8, 4, 2], f32, tag="mE")
            nc.vector.tensor_scalar_mul(out=mE, in0=pgv, scalar1=1.0 / 16.0)
            var_t = gnp.tile([8, 4], f32, tag="var")
            nc.vector.tensor_mul(out=var_t, in0=mE[:, :, 0], in1=mE[:, :, 0])
            nc.vector.tensor_sub(out=var_t, in0=mE[:, :, 1], in1=var_t)
            sd = gnp.tile([8, 4], f32, tag="sd")
            nc.scalar.activation(out=sd, in_=var_t, func=ACT.Sqrt,
                                 bias=eps_t[0:8, :], scale=1.0)
            m_rs = gnp.tile([8, 4, 2], f32r, tag="m_rs")
            nc.vector.tensor_copy(out=m_rs[:, :, 0], in_=mE[:, :, 0])
            with nc.allow_low_precision(reason="f32r rstd for matmul broadcast"):
                nc.vector.reciprocal(out=m_rs[:, :, 1], in_=sd)

            # h fp8 pair tiles [128, 2, 1024]; channel c = 256t + 128j + p
            ht = [hp.tile([128, 2, HW], fp8, tag=f"h{t}", name=f"h{t}")
                  for t in range(2)]
            for ct in range(NT):
                bc_ps = psB.tile([128, 1024], f32, tag="B", name=f"bc{ct}")
                nc.tensor.matmul(bc_ps[:, 0:2], lhsT=gbc_r, rhs=m_rs[:, ct, :],
                                 start=True, stop=True)
                mrt = gnp.tile([128, 2], f32, tag="mrt")
                nc.vector.tensor_copy(out=mrt, in_=bc_ps[:, 0:2])
                A_t = gnp.tile([128, 1], f32, tag=f"A{ct}", name=f"A{ct}")
                nc.vector.tensor_mul(out=A_t, in0=gamma_sb[:, ct:ct + 1],
                                     in1=mrt[:, 1:2])
                B_t = gnp.tile([128, 1], f32, tag=f"B{ct}", name=f"B{ct}")
                tmb = gnp.tile([128, 1], f32, tag="tmb")
                nc.vector.tensor_mul(out=tmb, in0=mrt[:, 0:1], in1=A_t)
                nc.vector.tensor_sub(out=B_t, in0=beta_sb[:, ct:ct + 1], in1=tmb)
                eng = nc.vector if ct % 2 == 0 else nc.gpsimd
                eng.tensor_scalar(
                    out=ht[ct // 2][:, ct % 2, :], in0=xt[ct],
                    scalar1=A_t, scalar2=B_t, op0=ALU.mult, op1=ALU.add)

            # ---------------- QK GEMM (fp8 DR) ----------------
            # W free layout: Q_h at 96*(2h) (65 rows: q+tq), K_h at
            # 96*(2h+1) (64 rows), aug tk at 1536 (40 rows, tk at 32-39).
            tkf = cp.tile([40, HW], f32, tag="tkf")
            blocks = []
            for h in range(NH):
                blocks.append((128 * (2 * h), 128, ("q", h)))
                blocks.append((128 * (2 * h + 1), 128, ("k", h)))
            blocks.append((2048, 40, ("aug", 0)))
            for off, rows, (kind, h) in blocks:
                ps = psA.tile([128, 1024], f32, tag="A", name=f"qk{kind}{h}")
                for t in range(2):
                    for n in range(2):
                        nc.tensor.matmul(
                            ps[0:rows, n * 512:(n + 1) * 512],
                            lhsT=wqk[t][:, :, off:off + rows],
                            rhs=ht[t][:, :, n * 512:(n + 1) * 512],
                            start=(t == 0), stop=(t == 1), perf_mode=DR)
                if kind == "q":
                    nc.vector.tensor_copy(out=QsB[:, h, :], in_=ps)
                elif kind == "k":
                    nc.scalar.copy(out=KsB[0:127, h, :], in_=ps[0:127, :])
                else:
                    nc.vector.tensor_copy(out=tkf[32:40, :], in_=ps[32:40, :])

            # tkT [128, 64]: tkT[p, kk*8+h] = tk_h[kk*128+p] via PE transpose
            tkr = cp.tile([40, HW], f32r, tag="tkr")
            nc.vector.tensor_copy(out=tkr[32:40, :], in_=tkf[32:40, :])
            tkps = psA.tile([128, 1024], f32, tag="A", name="tkps")
            for kk in range(NP):
                nc.tensor.matmul(
                    tkps[:, kk * 8:(kk + 1) * 8],
                    lhsT=tkr[32:40, kk * 128:(kk + 1) * 128],
                    rhs=i8r[32:40, :], start=True, stop=True)
            tkT = cp.tile([128, 64], f32, tag="tkT")
            nc.vector.tensor_copy(out=tkT, in_=tkps[:, 0:64])
            # exp biases: actB = SCALE*tk + (SCALE*bqbk - OFF)
            actB = cp.tile([128, 64], f32, tag="actB")
            nc.vector.scalar_tensor_tensor(out=actB, in0=tkT, scalar=SCALE,
                                           in1=cba_sb, op0=ALU.mult, op1=ALU.add)
            # Schraudolph bias: schB = A16*SCALE*tk + cbs
            schB = cp.tile([128, 64], f32, tag="schB")
            nc.vector.scalar_tensor_tensor(out=schB, in0=tkT, scalar=A16 * SCALE,
                                           in1=cbs_sb, op0=ALU.mult, op1=ALU.add)

            # ---------------- V GEMM (fp8 DR) ----------------
            for kk in range(NP):
                ps = psB.tile([128, 1024], f32, tag="B", name=f"v{kk}")
                for t in range(2):
                    nc.tensor.matmul(
                        ps[:, 0:512],
                        lhsT=ht[t][:, :, kk * 128:(kk + 1) * 128],
                        rhs=wv[t], start=(t == 0), stop=(t == 1), perf_mode=DR)
                nc.vector.tensor_copy(
                    out=vt[kk // 2][:, kk % 2, :, 0:HD],
                    in_=ps[:, 0:512].rearrange("p (h d) -> p h d", h=NH))

            # ---------------- attention ----------------
            attO = [aop.tile([128, 2, HW], fp8, tag=f"ao{tp}", name=f"ao{tp}")
                    for tp in range(4)]
            for tp in range(4):
                nc.gpsimd.memset(attO[tp][64:128, :, :], 0.0)

            def emit_scores(h):
                pps = []
                for kp in range(4):
                    pps.append(ppp.tile([128, 2, HW], fp8, tag="pp",
                                        name=f"pp{h}_{kp}"))
                pb3 = pbfp.tile([128, HW], bf16, tag="pb3", name=f"pb3{h}")
                for kk in range(NP):
                    ps = psA.tile([128, 1024], f32, tag="A", name=f"sc{h}_{kk}")
                    for n in range(2):
                        nc.tensor.matmul(
                            ps[:, n * 512:(n + 1) * 512],
                            lhsT=KsB[:, h, kk * 128:(kk + 1) * 128],
                            rhs=QsB[:, h, n * 512:(n + 1) * 512],
                            start=True, stop=True)
                    col = kk * 8 + h
                    if kk in EXP_DVE_KK:
                        # Schraudolph: int16 bits of bf16 exp approximation,
                        # then contiguous bf16 -> fp8 cast on GpSimd
                        nc.vector.tensor_scalar(
                            out=pb3.bitcast(mybir.dt.int16),
                            in0=ps, scalar1=A16 * SCALE,
                            scalar2=schB[:, col:col + 1],
                            op0=ALU.mult, op1=ALU.add)
                        nc.gpsimd.tensor_copy(
                            out=pps[kk // 2][:, kk % 2, :], in_=pb3)
                    else:
                        nc.scalar.activation(
                            out=pps[kk // 2][:, kk % 2, :], in_=ps, func=ACT.Exp,
                            bias=actB[:, col:col + 1], scale=SCALE)
                return pps

            def emit_av(h, pps):
                pa = psB.tile([128, 1024], f32, tag="B", name=f"pa{h}")
                for kp in range(4):
                    for n in range(2):
                        nc.tensor.matmul(
                            pa[0:HD + 1, n * 512:(n + 1) * 512],
                            lhsT=vt[kp][:, :, h, 0:HD + 1],
                            rhs=pps[kp][:, :, n * 512:(n + 1) * 512],
                            start=(kp == 0), stop=(kp == 3), perf_mode=DR)
                # evacuate unnormalized AV + denominator row to SBUF bf16
                paS = dsbp.tile([65, HW], bf16, tag="paS", name=f"paS{h}")
                with nc.allow_low_precision(reason="bf16 raw attn out"):
                    nc.vector.tensor_copy(out=paS, in_=pa[0:65, :])
                return paS

            def emit_norm(h, paS):
                rbps = psB.tile([128, 1024], f32, tag="B", name=f"rbps{h}")
                for n in range(2):
                    nc.tensor.matmul(
                        rbps[0:64, n * 512:(n + 1) * 512],
                        lhsT=o64b[64:65, :],
                        rhs=paS[64:65, n * 512:(n + 1) * 512],
                        start=True, stop=True)
                rb = rbp.tile([64, HW], f32, tag="rb", name=f"rb{h}")
                with nc.allow_low_precision(reason="recip of softmax denom"):
                    nc.vector.reciprocal_approx_fast(
                        out=rb, in_=rbps[0:64, :])
                nc.vector.tensor_mul(
                    out=attO[h // 2][0:64, h % 2, :], in0=paS[0:64, :], in1=rb)

            prev = emit_scores(0)
            pend = None
            for h in range(NH):
                nxt = emit_scores(h + 1) if h + 1 < NH else None
                cur = emit_av(h, prev)
                if pend is not None:
                    emit_norm(h - 1, pend)
                pend = cur
                prev = nxt
            emit_norm(NH - 1, pend)

            # ---------------- proj + residual ----------------
            for m in range(NT):
                ps = psA.tile([128, 1024], f32, tag="A", name=f"pr{m}")
                for tp in range(4):
                    for n in range(2):
                        nc.tensor.matmul(
                            ps[:, n * 512:(n + 1) * 512],
                            lhsT=wp[tp][:, :, m * 128:(m + 1) * 128],
                            rhs=attO[tp][:, :, n * 512:(n + 1) * 512],
                            start=(tp == 0), stop=(tp == 3), perf_mode=DR)
                nc.vector.scalar_tensor_tensor(
                    out=xt[m], in0=ps, scalar=wtld_sb[:, m:m + 1],
                    in1=xt[m], op0=ALU.add, op1=ALU.add)
                nc.sync.dma_start(out=out_d[m * 128:(m + 1) * 128, :], in_=xt[m])
    nc.compile()
    return nc


def _prep_common(gamma, beta, w_qkv, b_qkv, w_proj, b_proj):
    fp8np = ml_dtypes.float8_e4m3fn
    w_qkv = np.asarray(w_qkv, np.float32)
    w_proj = np.asarray(w_proj, np.float32)
    b_qkv = np.asarray(b_qkv, np.float32)
    b_proj = np.asarray(b_proj, np.float32)
    wq, wk, wvm = w_qkv[0:C], w_qkv[C:2 * C], w_qkv[2 * C:3 * C]
    bq, bk, bv = b_qkv[0:C], b_qkv[C:2 * C], b_qkv[2 * C:3 * C]

    # W free layout per head: Q-block [q(64); 0(63); tq(1)] at 128*(2h),
    # K-block [k(64); 0(64)] at 128*(2h+1); aug tk rows 32-39 at 2048.
    Wqk = np.zeros((2112, C), np.float32)
    for h in range(NH):
        sl = slice(h * HD, (h + 1) * HD)
        qo = 128 * (2 * h)
        ko = 128 * (2 * h + 1)
        Wqk[qo:qo + 64] = wq[sl]
        Wqk[qo + 127] = bk[sl] @ wq[sl]
        Wqk[ko:ko + 64] = wk[sl]
        Wqk[2048 + 32 + h] = bq[sl] @ wk[sl]
    WqkT = Wqk.T  # [512 c, 2112]
    wqk_dr = np.stack([
        WqkT[256 * t:256 * (t + 1)].reshape(2, 128, 2112).transpose(1, 0, 2)
        for t in range(2)])  # [2, 128, 2, 2112]

    WvT = wvm.T  # [512 c, 512 oc]
    wv_dr = np.stack([
        WvT[256 * t:256 * (t + 1)].reshape(2, 128, 512).transpose(1, 0, 2)
        for t in range(2)])

    WpT = w_proj.T  # [512 c, 512 m]
    wp_dr = np.zeros((4, 128, 2, 512), np.float32)
    for tp in range(4):
        wp_dr[tp, 0:64] = (
            WpT[128 * tp:128 * (tp + 1)].reshape(2, 64, 512).transpose(1, 0, 2))

    wtld = w_proj @ bv + b_proj

    bqbk = np.array([bq[h * HD:(h + 1) * HD] @ bk[h * HD:(h + 1) * HD]
                     for h in range(NH)], np.float32)
    cba = np.tile(SCALE * bqbk - OFF, 8).astype(np.float32)
    cbs = np.tile(A16 * (SCALE * bqbk - OFF) + (B16 - CCOR),
                  8).astype(np.float32)

    cns8 = np.zeros((2, 1024), fp8np)
    cns8[0] = fp8np(1.0)

    return {
        "wqk": np.ascontiguousarray(wqk_dr.astype(fp8np)),
        "wv8": np.ascontiguousarray(wv_dr.astype(fp8np)),
        "wp8": np.ascontiguousarray(wp_dr.astype(fp8np)),
        "wtld": np.ascontiguousarray(wtld.astype(np.float32)),
        "gamma": np.ascontiguousarray(np.asarray(gamma, np.float32)),
        "beta": np.ascontiguousarray(np.asarray(beta, np.float32)),
        "gind": np.ascontiguousarray(
            np.repeat(np.eye(8, dtype=np.float32), 16, axis=0)),
        "gbc": np.ascontiguousarray(
            np.repeat(np.eye(8, dtype=np.float32), 16, axis=1)),
        "cba": cba,
        "cbs": cbs,
        "cns8": cns8,
        "cnsb": np.concatenate([np.ones((1, 1024), ml_dtypes.bfloat16),
                                np.zeros((1, 1024), ml_dtypes.bfloat16)]),
        "onesr": np.ones((1, 8192), ml_dtypes.bfloat16),
        "eye8": np.eye(8, dtype=np.float32),
        "ones64": np.ones((1, 64), np.float32),
    }


_NC = None


def kernel(x, gamma, beta, w_qkv, b_qkv, w_proj, b_proj):
    global _NC
    x = np.asarray(x, dtype=np.float32)
    B = x.shape[0]
    assert B == 8
    if _NC is None:
        _NC = _build()
    common = _prep_common(gamma, beta, w_qkv, b_qkv, w_proj, b_proj)
    in_maps = [
        {"x": np.ascontiguousarray(x[b].reshape(C, HW)), **common}
        for b in range(B)
    ]
    res = run_bass_kernel_spmd(_NC, in_maps, core_ids=list(range(8)))
    out = np.stack([res.results[b]["out"] for b in range(B)])
    return out.reshape(B, C, 32, 32).astype(np.float32)


# revision 50
# speedup vs baseline: 1.1758x; 1.0021x over previous
"""AttentionBlock Trainium2 kernel (v2: fp8 DoubleRow GEMMs).

Sharding: data-parallel over batch (B=8 -> one batch element per NeuronCore).

Per-core pipeline (C=512, HW=1024, 8 heads x 64):
  groupnorm (stats via indicator matmuls, f32)
  -> h cast to fp8 pair-tile layout [128,2,1024]
  -> QK GEMM in fp8 DoubleRow (2x MACs/cycle); biases algebraically removed:
       score bias terms ride as (a) one augmented contraction channel
       (tq = bk_h^T W_q,h h, paired with a constant-1 channel on the K side)
       and (b) a per-partition exp bias (tk = bq_h^T W_k,h h + bq.bk).
  -> Q,K shuffled to [33,2,1024] fp8 DR layout by on-chip DMA (c = 2p+j)
  -> scores GEMM fp8 DR, transposed [k,q]; exp with offset OFF so probs
     fit fp8e4 (max 240); denominators ride as a ones-column in the
     attn@v DR matmul; exp split: ScalarE native exp -> fp8, VectorE
     Schraudolph bit-trick (magic-add) -> bf16 bits -> GpSimd cast fp8.
  -> AV GEMM fp8 DR over kk-pairs -> normalize via reciprocal_approx_fast
     + DMA broadcast -> attO fp8 [64,2,1024] head-pair tiles
  -> proj GEMM fp8 DR + residual (+ W_p b_v + b_proj folded per-channel).
"""
import sys

sys.path.insert(0, "/opt/trn_rl_repo")
import numpy as np
import ml_dtypes
import concourse.bass as bass
import concourse.bacc as bacc
import concourse.tile as tile
from concourse import mybir
from concourse.bass_utils import run_bass_kernel_spmd

f32 = mybir.dt.float32
f32r = mybir.dt.float32r
bf16 = mybir.dt.bfloat16
fp8 = mybir.dt.float8e4
ALU = mybir.AluOpType
ACT = mybir.ActivationFunctionType
DR = mybir.MatmulPerfMode.DoubleRow

C = 512
HW = 1024
NH = 8
HD = 64
EPS = 1e-5
SCALE = HD ** -0.5
NT = C // 128  # 4 channel tiles
NP = HW // 128  # 8 position tiles

OFF = 2.0          # exp offset: probs = exp(s*SCALE - OFF)
A16 = 128.0 / np.log(2.0)   # Schraudolph slope for bf16 bit pattern
B16 = 127 * 128             # bf16 exponent bias << 7
CCOR = 9.0                  # Schraudolph bias correction
MAGIC = float(2 ** 23)
# kk tiles whose exp runs on DVE (Schraudolph) instead of ScalarE
EXP_DVE_KK = ()


def _build():
    nc = bacc.Bacc("TRN2", target_bir_lowering=False, debug=False, num_devices=8)
    x_d = nc.dram_tensor("x", [C, HW], f32, kind="ExternalInput").ap()
    wqk_d = nc.dram_tensor("wqk", [2, 128, 2, 2112], fp8, kind="ExternalInput").ap()
    cnsb_d = nc.dram_tensor("cnsb", [2, 1024], bf16, kind="ExternalInput").ap()
    onesr_d = nc.dram_tensor("onesr", [1, 8192], bf16, kind="ExternalInput").ap()
    wv_d = nc.dram_tensor("wv8", [2, 128, 2, 512], fp8, kind="ExternalInput").ap()
    wp_d = nc.dram_tensor("wp8", [4, 128, 2, 512], fp8, kind="ExternalInput").ap()
    wtld_d = nc.dram_tensor("wtld", [C], f32, kind="ExternalInput").ap()
    gamma_d = nc.dram_tensor("gamma", [C], f32, kind="ExternalInput").ap()
    beta_d = nc.dram_tensor("beta", [C], f32, kind="ExternalInput").ap()
    gind_d = nc.dram_tensor("gind", [128, 8], f32, kind="ExternalInput").ap()
    gbc_d = nc.dram_tensor("gbc", [8, 128], f32, kind="ExternalInput").ap()
    cba_d = nc.dram_tensor("cba", [64], f32, kind="ExternalInput").ap()
    cbs_d = nc.dram_tensor("cbs", [64], f32, kind="ExternalInput").ap()
    cns8_d = nc.dram_tensor("cns8", [2, 1024], fp8, kind="ExternalInput").ap()
    out_d = nc.dram_tensor("out", [C, HW], f32, kind="ExternalOutput").ap()
    eye8_d = nc.dram_tensor("eye8", [8, 8], f32, kind="ExternalInput").ap()
    ones64_d = nc.dram_tensor("ones64", [1, 64], f32, kind="ExternalInput").ap()
    d_scr = nc.dram_tensor("d_scr", [NH, HW], f32)

    with tile.TileContext(nc) as tc:
        with (
            tc.tile_pool(name="const", bufs=1) as cp,
            tc.tile_pool(name="gnp", bufs=2) as gnp,
            tc.tile_pool(name="xp", bufs=1) as xp,
            tc.tile_pool(name="hp", bufs=1) as hp,
            tc.tile_pool(name="qk8", bufs=1) as qk8p,
            tc.tile_pool(name="qs", bufs=1) as qsp,
            tc.tile_pool(name="vt", bufs=1) as vtp,
            tc.tile_pool(name="pp", bufs=8) as ppp,
            tc.tile_pool(name="pbf", bufs=2) as pbfp,
            tc.tile_pool(name="dsb", bufs=3) as dsbp,
            tc.tile_pool(name="rb", bufs=2) as rbp,
            tc.tile_pool(name="ao", bufs=1) as aop,
            tc.tile_pool(name="psA", bufs=2, space="PSUM") as psA,
            tc.tile_pool(name="psB", bufs=2, space="PSUM") as psB,
        ):
            # ---------------- loads ----------------
            xt = []
            for t in range(NT):
                xi = xp.tile([128, HW], f32, tag=f"x{t}")
                nc.sync.dma_start(out=xi, in_=x_d[t * 128:(t + 1) * 128, :])
                xt.append(xi)

            wqk = []
            for t in range(2):
                w = cp.tile([128, 2, 2112], fp8, tag=f"wqk{t}")
                nc.sync.dma_start(out=w, in_=wqk_d[t])
                wqk.append(w)
            wv = []
            for t in range(2):
                w = cp.tile([128, 2, 512], fp8, tag=f"wv{t}")
                nc.sync.dma_start(out=w, in_=wv_d[t])
                wv.append(w)
            wp = []
            for t in range(4):
                w = cp.tile([128, 2, 512], fp8, tag=f"wp{t}")
                nc.sync.dma_start(out=w, in_=wp_d[t])
                wp.append(w)

            def col_load(src_ap, offset, name):
                t_ = cp.tile([128, NT], f32, tag=name)
                nc.sync.dma_start(
                    out=t_,
                    in_=bass.AP(tensor=src_ap.tensor, offset=offset,
                                ap=[[1, 128], [128, NT]]),
                )
                return t_

            gamma_sb = col_load(gamma_d, 0, "gamma")
            beta_sb = col_load(beta_d, 0, "beta")
            wtld_sb = col_load(wtld_d, 0, "wtld")

            cba_sb = cp.tile([128, 64], f32, tag="cba")
            nc.sync.dma_start(
                out=cba_sb,
                in_=bass.AP(tensor=cba_d.tensor, offset=0, ap=[[0, 128], [1, 64]]))
            cbs_sb = cp.tile([128, 64], f32, tag="cbs")
            nc.sync.dma_start(
                out=cbs_sb,
                in_=bass.AP(tensor=cbs_d.tensor, offset=0, ap=[[0, 128], [1, 64]]))

            o64b = cp.tile([65, 64], bf16, tag="o64b")
            nc.sync.dma_start(out=o64b[64:65, :], in_=cnsb_d[0:1, 0:64])
            i8f = cp.tile([72, 8], f32, tag="i8f")
            nc.sync.dma_start(out=i8f[64:72, :], in_=eye8_d)
            gind_f = cp.tile([128, 8], f32, tag="gindf")
            nc.sync.dma_start(out=gind_f, in_=gind_d)
            gbc_f = cp.tile([8, 128], f32, tag="gbcf")
            nc.sync.dma_start(out=gbc_f, in_=gbc_d)

            # Q/K bf16 scores operands [rows, head, pos], 128-row padded
            # so the scores contraction lights the full PE array (HAM warm).
            # QsB rows: q(0-63), zeros(64-126), tq(127);
            # KsB rows: k(0-63), zeros(64-126), ones(127).
            QsB = qsp.tile([128, NH, HW], bf16, tag="QsB")
            KsB = qsp.tile([128, NH, HW], bf16, tag="KsB")
            nc.sync.dma_start(out=KsB[127:128, :, :], in_=onesr_d)

            # vT tiles per kk-pair: [128, 2, NH, 96-padded]; col 64 = ones.
            ones16f = cp.tile([128, 16], f32, tag="ones16f")
            nc.gpsimd.memset(ones16f, 1.0)
            vt = []
            for kp in range(4):
                v = vtp.tile([128, 2, NH, 96], fp8, tag=f"vt{kp}")
                nc.vector.tensor_copy(
                    out=v[:, :, :, HD:HD + 1].rearrange("p a b c -> p (a b c)"),
                    in_=ones16f)
                vt.append(v)

            # ---------------- groupnorm ----------------
            eps_t = cp.tile([128, 1], f32, tag="eps")
            nc.vector.memset(eps_t, EPS)
            pg = psB.tile([128, 1024], f32, tag="B", name="pg")
            mes = []
            for t in range(NT):
                st = gnp.tile([128, 2, 6], f32, tag="bnst")
                nc.vector.bn_stats(out=st[:, 0, :], in_=xt[t][:, 0:512])
                nc.vector.bn_stats(out=st[:, 1, :], in_=xt[t][:, 512:1024])
                mv = gnp.tile([128, 2], f32, tag="mv")
                nc.vector.bn_aggr(out=mv, in_=st)
                me = gnp.tile([128, 2], f32r, tag=f"me{t}", name=f"me{t}")
                nc.vector.tensor_copy(out=me[:, 0:1], in_=mv[:, 0:1])
                sq = gnp.tile([128, 1], f32, tag="sq")
                nc.vector.tensor_mul(out=sq, in0=mv[:, 0:1], in1=mv[:, 0:1])
                nc.vector.tensor_add(out=me[:, 1:2], in0=mv[:, 1:2], in1=sq)
                mes.append(me)
            gind_r = cp.tile([128, 8], f32r, tag="gindr")
            nc.vector.tensor_copy(out=gind_r, in_=gind_f)
            gbc_r = cp.tile([8, 128], f32r, tag="gbcr")
            nc.vector.tensor_copy(out=gbc_r, in_=gbc_f)
            i8r = cp.tile([72, 8], f32r, tag="i8r")
            nc.vector.tensor_copy(out=i8r[64:72, :], in_=i8f[64:72, :])
            for t in range(NT):
                nc.tensor.matmul(
                    pg[0:8, 0:8].rearrange("p (a b) -> p a b", b=2)[:, t, :],
                    lhsT=gind_r, rhs=mes[t], start=True, stop=True)
            pgv = pg[0:8, 0:8].rearrange("p (a b) -> p a b", b=2)
            mE = gnp.tile([8, 4, 2], f32, tag="mE")
            nc.vector.tensor_scalar_mul(out=mE, in0=pgv, scalar1=1.0 / 16.0)
            var_t = gnp.tile([8, 4], f32, tag="var")
            nc.vector.tensor_mul(out=var_t, in0=mE[:, :, 0], in1=mE[:, :, 0])
            nc.vector.tensor_sub(out=var_t, in0=mE[:, :, 1], in1=var_t)
            sd = gnp.tile([8, 4], f32, tag="sd")
            nc.scalar.activation(out=sd, in_=var_t, func=ACT.Sqrt,
                                 bias=eps_t[0:8, :], scale=1.0)
            m_rs = gnp.tile([8, 4, 2], f32r, tag="m_rs")
            nc.vector.tensor_copy(out=m_rs[:, :, 0], in_=mE[:, :, 0])
            with nc.allow_low_precision(reason="f32r rstd for matmul broadcast"):
                nc.vector.reciprocal(out=m_rs[:, :, 1], in_=sd)

            # h fp8 pair tiles [128, 2, 1024]; channel c = 256t + 128j + p
            ht = [hp.tile([128, 2, HW], fp8, tag=f"h{t}", name=f"h{t}")
                  for t in range(2)]
            for ct in range(NT):
                bc_ps = psB.tile([128, 1024], f32, tag="B", name=f"bc{ct}")
                nc.tensor.matmul(bc_ps[:, 0:2], lhsT=gbc_r, rhs=m_rs[:, ct, :],
                                 start=True, stop=True)
                mrt = gnp.tile([128, 2], f32, tag="mrt")
                nc.vector.tensor_copy(out=mrt, in_=bc_ps[:, 0:2])
                A_t = gnp.tile([128, 1], f32, tag=f"A{ct}", name=f"A{ct}")
                nc.vector.tensor_mul(out=A_t, in0=gamma_sb[:, ct:ct + 1],
                                     in1=mrt[:, 1:2])
                B_t = gnp.tile([128, 1], f32, tag=f"B{ct}", name=f"B{ct}")
                tmb = gnp.tile([128, 1], f32, tag="tmb")
                nc.vector.tensor_mul(out=tmb, in0=mrt[:, 0:1], in1=A_t)
                nc.vector.tensor_sub(out=B_t, in0=beta_sb[:, ct:ct + 1], in1=tmb)
                eng = nc.vector if ct % 2 == 0 else nc.gpsimd
                eng.tensor_scalar(
                    out=ht[ct // 2][:, ct % 2, :], in0=xt[ct],
                    scalar1=A_t, scalar2=B_t, op0=ALU.mult, op1=ALU.add)

            # ---------------- QK GEMM (fp8 DR) ----------------
            # W free layout: Q_h at 96*(2h) (65 rows: q+tq), K_h at
            # 96*(2h+1) (64 rows), aug tk at 1536 (40 rows, tk at 32-39).
            tkf = cp.tile([72, HW], f32, tag="tkf")
            blocks = []
            for h in range(NH):
                blocks.append((128 * (2 * h), 128, ("q", h)))
                blocks.append((128 * (2 * h + 1), 128, ("k", h)))

            for off, rows, (kind, h) in blocks:
                ps = psA.tile([128, 1024], f32, tag="A", name=f"qk{kind}{h}")
                for t in range(2):
                    for n in range(2):
                        nc.tensor.matmul(
                            ps[0:rows, n * 512:(n + 1) * 512],
                            lhsT=wqk[t][:, :, off:off + rows],
                            rhs=ht[t][:, :, n * 512:(n + 1) * 512],
                            start=(t == 0), stop=(t == 1), perf_mode=DR)
                if kind == "q":
                    nc.vector.tensor_copy(out=QsB[:, h, :], in_=ps)
                else:
                    nc.scalar.copy(out=KsB[0:127, h, :], in_=ps[0:127, :])
                    if h == 0:
                        nc.vector.tensor_copy(out=tkf[64:72, :],
                                              in_=ps[64:72, :])

            # tkT [128, 64]: tkT[p, kk*8+h] = tk_h[kk*128+p] via PE transpose
            tkr = cp.tile([72, HW], f32r, tag="tkr")
            nc.vector.tensor_copy(out=tkr[64:72, :], in_=tkf[64:72, :])
            tkps = psA.tile([128, 1024], f32, tag="A", name="tkps")
            for kk in range(NP):
                nc.tensor.matmul(
                    tkps[:, kk * 8:(kk + 1) * 8],
                    lhsT=tkr[64:72, kk * 128:(kk + 1) * 128],
                    rhs=i8r[64:72, :], start=True, stop=True)
            tkT = cp.tile([128, 64], f32, tag="tkT")
            nc.vector.tensor_copy(out=tkT, in_=tkps[:, 0:64])
            # exp biases: actB = SCALE*tk + (SCALE*bqbk - OFF)
            actB = cp.tile([128, 64], f32, tag="actB")
            nc.vector.scalar_tensor_tensor(out=actB, in0=tkT, scalar=SCALE,
                                           in1=cba_sb, op0=ALU.mult, op1=ALU.add)
            # Schraudolph bias: schB = A16*SCALE*tk + cbs
            schB = cp.tile([128, 64], f32, tag="schB")
            nc.vector.scalar_tensor_tensor(out=schB, in0=tkT, scalar=A16 * SCALE,
                                           in1=cbs_sb, op0=ALU.mult, op1=ALU.add)

            # ---------------- V GEMM (fp8 DR) ----------------
            for kk in range(NP):
                ps = psB.tile([128, 1024], f32, tag="B", name=f"v{kk}")
                for t in range(2):
                    nc.tensor.matmul(
                        ps[:, 0:512],
                        lhsT=ht[t][:, :, kk * 128:(kk + 1) * 128],
                        rhs=wv[t], start=(t == 0), stop=(t == 1), perf_mode=DR)
                nc.vector.tensor_copy(
                    out=vt[kk // 2][:, kk % 2, :, 0:HD],
                    in_=ps[:, 0:512].rearrange("p (h d) -> p h d", h=NH))

            # ---------------- attention ----------------
            attO = [aop.tile([128, 2, HW], fp8, tag=f"ao{tp}", name=f"ao{tp}")
                    for tp in range(4)]
            for tp in range(4):
                nc.gpsimd.memset(attO[tp][64:128, :, :], 0.0)

            def emit_scores(h):
                pps = []
                for kp in range(4):
                    pps.append(ppp.tile([128, 2, HW], fp8, tag="pp",
                                        name=f"pp{h}_{kp}"))
                pb3 = pbfp.tile([128, HW], bf16, tag="pb3", name=f"pb3{h}")
                for kk in range(NP):
                    ps = psA.tile([128, 1024], f32, tag="A", name=f"sc{h}_{kk}")
                    for n in range(2):
                        nc.tensor.matmul(
                            ps[:, n * 512:(n + 1) * 512],
                            lhsT=KsB[:, h, kk * 128:(kk + 1) * 128],
                            rhs=QsB[:, h, n * 512:(n + 1) * 512],
                            start=True, stop=True)
                    col = kk * 8 + h
                    if kk in EXP_DVE_KK:
                        # Schraudolph: int16 bits of bf16 exp approximation,
                        # then contiguous bf16 -> fp8 cast on GpSimd
                        nc.vector.tensor_scalar(
                            out=pb3.bitcast(mybir.dt.int16),
                            in0=ps, scalar1=A16 * SCALE,
                            scalar2=schB[:, col:col + 1],
                            op0=ALU.mult, op1=ALU.add)
                        nc.gpsimd.tensor_copy(
                            out=pps[kk // 2][:, kk % 2, :], in_=pb3)
                    else:
                        nc.scalar.activation(
                            out=pps[kk // 2][:, kk % 2, :], in_=ps, func=ACT.Exp,
                            bias=actB[:, col:col + 1], scale=SCALE)
                return pps

            def emit_av(h, pps):
                pa = psB.tile([128, 1024], f32, tag="B", name=f"pa{h}")
                for kp in range(4):
                    for n in range(2):
                        nc.tensor.matmul(
                            pa[0:HD + 1, n * 512:(n + 1) * 512],
                            lhsT=vt[kp][:, :, h, 0:HD + 1],
                            rhs=pps[kp][:, :, n * 512:(n + 1) * 512],
                            start=(kp == 0), stop=(kp == 3), perf_mode=DR)
                # evacuate unnormalized AV + denominator row to SBUF bf16
                paS = dsbp.tile([65, HW], bf16, tag="paS", name=f"paS{h}")
                with nc.allow_low_precision(reason="bf16 raw attn out"):
                    nc.vector.tensor_copy(out=paS, in_=pa[0:65, :])
                return paS

            def emit_norm(h, paS):
                rbps = psB.tile([128, 1024], f32, tag="B", name=f"rbps{h}")
                for n in range(2):
                    nc.tensor.matmul(
                        rbps[0:64, n * 512:(n + 1) * 512],
                        lhsT=o64b[64:65, :],
                        rhs=paS[64:65, n * 512:(n + 1) * 512],
                        start=True, stop=True)
                rb = rbp.tile([64, HW], f32, tag="rb", name=f"rb{h}")
                with nc.allow_low_precision(reason="recip of softmax denom"):
                    nc.vector.reciprocal_approx_fast(
                        out=rb, in_=rbps[0:64, :])
                nc.vector.tensor_mul(
                    out=attO[h // 2][0:64, h % 2, :], in0=paS[0:64, :], in1=rb)

            prev = emit_scores(0)
            pend = None
            for h in range(NH):
                nxt = emit_scores(h + 1) if h + 1 < NH else None
                cur = emit_av(h, prev)
                if pend is not None:
                    emit_norm(h - 1, pend)
                pend = cur
                prev = nxt
            emit_norm(NH - 1, pend)

            # ---------------- proj + residual ----------------
            for m in range(NT):
                ps = psA.tile([128, 1024], f32, tag="A", name=f"pr{m}")
                for tp in range(4):
                    for n in range(2):
                        nc.tensor.matmul(
                            ps[:, n * 512:(n + 1) * 512],
                            lhsT=wp[tp][:, :, m * 128:(m + 1) * 128],
                            rhs=attO[tp][:, :, n * 512:(n + 1) * 512],
                            start=(tp == 0), stop=(tp == 3), perf_mode=DR)
                nc.vector.scalar_tensor_tensor(
                    out=xt[m], in0=ps, scalar=wtld_sb[:, m:m + 1],
                    in1=xt[m], op0=ALU.add, op1=ALU.add)
                nc.sync.dma_start(out=out_d[m * 128:(m + 1) * 128, :], in_=xt[m])
    nc.compile()
    return nc


def _prep_common(gamma, beta, w_qkv, b_qkv, w_proj, b_proj):
    fp8np = ml_dtypes.float8_e4m3fn
    w_qkv = np.asarray(w_qkv, np.float32)
    w_proj = np.asarray(w_proj, np.float32)
    b_qkv = np.asarray(b_qkv, np.float32)
    b_proj = np.asarray(b_proj, np.float32)
    wq, wk, wvm = w_qkv[0:C], w_qkv[C:2 * C], w_qkv[2 * C:3 * C]
    bq, bk, bv = b_qkv[0:C], b_qkv[C:2 * C], b_qkv[2 * C:3 * C]

    # W free layout per head: Q-block [q(64); 0(63); tq(1)] at 128*(2h),
    # K-block [k(64); 0(64)] at 128*(2h+1); aug tk rows 32-39 at 2048.
    Wqk = np.zeros((2112, C), np.float32)
    for h in range(NH):
        sl = slice(h * HD, (h + 1) * HD)
        qo = 128 * (2 * h)
        ko = 128 * (2 * h + 1)
        Wqk[qo:qo + 64] = wq[sl]
        Wqk[qo + 127] = bk[sl] @ wq[sl]
        Wqk[ko:ko + 64] = wk[sl]
        Wqk[128 + 64 + h] = bq[sl] @ wk[sl]
    WqkT = Wqk.T  # [512 c, 2112]
    wqk_dr = np.stack([
        WqkT[256 * t:256 * (t + 1)].reshape(2, 128, 2112).transpose(1, 0, 2)
        for t in range(2)])  # [2, 128, 2, 2112]

    WvT = wvm.T  # [512 c, 512 oc]
    wv_dr = np.stack([
        WvT[256 * t:256 * (t + 1)].reshape(2, 128, 512).transpose(1, 0, 2)
        for t in range(2)])

    WpT = w_proj.T  # [512 c, 512 m]
    wp_dr = np.zeros((4, 128, 2, 512), np.float32)
    for tp in range(4):
        wp_dr[tp, 0:64] = (
            WpT[128 * tp:128 * (tp + 1)].reshape(2, 64, 512).transpose(1, 0, 2))

    wtld = w_proj @ bv + b_proj

    bqbk = np.array([bq[h * HD:(h + 1) * HD] @ bk[h * HD:(h + 1) * HD]
                     for h in range(NH)], np.float32)
    cba = np.tile(SCALE * bqbk - OFF, 8).astype(np.float32)
    cbs = np.tile(A16 * (SCALE * bqbk - OFF) + (B16 - CCOR),
                  8).astype(np.float32)

    cns8 = np.zeros((2, 1024), fp8np)
    cns8[0] = fp8np(1.0)

    return {
        "wqk": np.ascontiguousarray(wqk_dr.astype(fp8np)),
        "wv8": np.ascontiguousarray(wv_dr.astype(fp8np)),
        "wp8": np.ascontiguousarray(wp_dr.astype(fp8np)),
        "wtld": np.ascontiguousarray(wtld.astype(np.float32)),
        "gamma": np.ascontiguousarray(np.asarray(gamma, np.float32)),
        "beta": np.ascontiguousarray(np.asarray(beta, np.float32)),
        "gind": np.ascontiguousarray(
            np.repeat(np.eye(8, dtype=np.float32), 16, axis=0)),
        "gbc": np.ascontiguousarray(
            np.repeat(np.eye(8, dtype=np.float32), 16, axis=1)),
        "cba": cba,
        "cbs": cbs,
        "cns8": cns8,
        "cnsb": np.concatenate([np.ones((1, 1024), ml_dtypes.bfloat16),
                                np.zeros((1, 1024), ml_dtypes.bfloat16)]),
        "onesr": np.ones((1, 8192), ml_dtypes.bfloat16),
        "eye8": np.eye(8, dtype=np.float32),
        "ones64": np.ones((1, 64), np.float32),
    }


_NC = None


def kernel(x, gamma, beta, w_qkv, b_qkv, w_proj, b_proj):
    global _NC
    x = np.asarray(x, dtype=np.float32)
    B = x.shape[0]
    assert B == 8
    if _NC is None:
        _NC = _build()
    common = _prep_common(gamma, beta, w_qkv, b_qkv, w_proj, b_proj)
    in_maps = [
        {"x": np.ascontiguousarray(x[b].reshape(C, HW)), **common}
        for b in range(B)
    ]
    res = run_bass_kernel_spmd(_NC, in_maps, core_ids=list(range(8)))
    out = np.stack([res.results[b]["out"] for b in range(B)])
    return out.reshape(B, C, 32, 32).astype(np.float32)
